# revision 28
# baseline (speedup 1.0000x reference)
"""Guided attention kernel for Trainium2, 8-core data-parallel over batch.

Math per batch b (C=64, D=8, N=H*W=4096):
  q = Wq @ query            [D, N]   (biases are zero in this problem)
  k = Wk @ query            [D, N]
  v = Wv @ value            [C, N]
  E[n, m] = sum_d q[d, n] k[d, m]
  A = softmax_m(E)
  out[c, n] = sum_m v[c, m] A[n, m] + value[c, n]

Device strategy (one batch per NeuronCore):
  - Host augments inputs: xq = [query; 1] (65, N), xv = [value; 1] (65, N),
    gt = wq_aug @ wk_aug^T (65, 65), wv (65, 128) with Wv^T in columns
    64..127 and a ones column at 0 so the output matmul's PSUM row 0 is the
    softmax denominator.
  - Energy computed transposed E^T = xq^T (gt^T xq) on the PE; exp feeds the
    output matmul as the moving operand.
  - exp is split across TWO engines: ACT does exact spline exp on 2 of every
    3 m-chunks; the DVE computes the third with a Schraudolph bit-trick --
    one tensor_scalar (y = E*128*log2e + bias) with int16 output dtype whose
    bit pattern IS the bf16 exp approximation (rel err ~3%, which the softmax
    normalization mostly cancels; measured end-to-end rel err 0.015 vs the
    2e-2 gate). This relieves the ACT engine, the steady-state bottleneck.
  - Epilogue mul/add run on the (otherwise idle) GPSIMD so the reciprocal
    broadcast's DRAM-bounce latency cannot head-of-line-block the DVE queue.
  - PSUM: 2x3-bank energy ping-pong + 1 output accumulator bank + 1
    projection bank (vt/kg projections are emitted just-in-time between
    rounds, overlapping the input DMA instead of serializing before it).
  - Input DMA in 512-col chunks, xq front-loaded, split across both HWDGE
    rings; a throwaway exp preloads the ACT spline table during the DMA.
"""

import sys

sys.path.insert(0, "/opt/trn_rl_repo")

import ml_dtypes
import numpy as np

import concourse.bacc as bacc
import concourse.bass as bass
import concourse.tile as tile
from concourse import mybir
from concourse.bass_utils import run_bass_kernel_spmd

F32 = mybir.dt.float32
BF16 = mybir.dt.bfloat16
I16 = mybir.dt.int16
EXP = mybir.ActivationFunctionType.Exp
MULT = mybir.AluOpType.mult
ADD = mybir.AluOpType.add

C = 64
CH = 65             # augmented channels (64 + ones row)
N = 4096
NG = 512            # n-group width (columns per psum bank)
NGROUPS = N // NG   # 8
MC = 128            # m-chunk width
MCHUNKS = N // MC   # 32
XCH = 512           # input dma chunk width
RSZ = 3
MGROUPS = [3] * 10 + [2]   # m-chunks per round; 11 rounds per group

LOG2E = 1.4426950408889634
EXPA = 128.0 * LOG2E              # bf16 exponent-grid scale
EXPB = 127.0 * 128.0 - 5.57       # bias incl. Schraudolph centering

TRACE = False
_CACHE = {}


def build_program():
    nc = bacc.Bacc("TRN2", debug=False)

    xq_d = nc.dram_tensor("xq", [CH, N], BF16, kind="ExternalInput")
    xv_d = nc.dram_tensor("xv", [CH, N], BF16, kind="ExternalInput")
    gt_d = nc.dram_tensor("gt", [CH, CH], BF16, kind="ExternalInput")
    wv_d = nc.dram_tensor("wv", [CH, MC], BF16, kind="ExternalInput")
    ones_d = nc.dram_tensor("ones", [1, C], BF16, kind="ExternalInput")
    out_d = nc.dram_tensor("out", [C, N], F32, kind="ExternalOutput")
    rec_d = nc.dram_tensor("recscratch", [NGROUPS, NG], F32, kind="Internal")

    with (
        tile.TileContext(nc) as tc,
        tc.tile_pool(name="consts", bufs=1) as consts,
        tc.tile_pool(name="expp", bufs=3) as expp,
        tc.tile_pool(name="expd", bufs=3) as expd,
        tc.tile_pool(name="small", bufs=2) as small,
        tc.tile_pool(name="pa_ps", bufs=2, space="PSUM") as pa_ps,
        tc.tile_pool(name="pd_ps", bufs=2, space="PSUM") as pd_ps,
        tc.tile_pool(name="po_ps", bufs=1, space="PSUM") as po_ps,
        tc.tile_pool(name="pr_ps", bufs=1, space="PSUM") as pr_ps,
    ):
        xq_sb = consts.tile([CH, N], BF16)
        xv_sb = consts.tile([CH, N], BF16)
        kg_sb = consts.tile([CH, N], BF16)
        vt_sb = consts.tile([MC, N], BF16)
        out_sb = consts.tile([C, N], F32)
        gt_sb = consts.tile([CH, CH], BF16)
        wv_sb = consts.tile([CH, MC], BF16)
        ones_sb = consts.tile([1, C], BF16)
        warm_sb = consts.tile([1, C], BF16)

        # --- input DMA: tiny consts, then 512-col chunks on both HWDGE
        # rings; xq front-loaded (energy critical path), xv interleaved so
        # vt quad j is projectable before its first AV use ---
        nc.sync.dma_start(out=gt_sb, in_=gt_d[:])

        def in_chunk(eng, t_d, t_sb, j):
            cols = slice(j * XCH, (j + 1) * XCH)
            eng.dma_start(out=t_sb[:, cols], in_=t_d[:, cols])

        # xq chunk 0 rides first on the scalar ring, in parallel with gt on
        # the sync ring -- together they unblock kg0 (the critical path)
        in_chunk(nc.scalar, xq_d, xq_sb, 0)
        nc.scalar.dma_start(out=wv_sb, in_=wv_d[:])
        nc.scalar.dma_start(out=ones_sb, in_=ones_d[:])

        for eng, t_d, t_sb, j in [
            (nc.sync, xq_d, xq_sb, 1),
            (nc.scalar, xq_d, xq_sb, 2), (nc.sync, xv_d, xv_sb, 0),
            (nc.scalar, xv_d, xv_sb, 1), (nc.sync, xq_d, xq_sb, 3),
            (nc.sync, xv_d, xv_sb, 2), (nc.scalar, xq_d, xq_sb, 4),
            (nc.sync, xq_d, xq_sb, 5), (nc.scalar, xq_d, xq_sb, 6),
            (nc.sync, xq_d, xq_sb, 7), (nc.scalar, xv_d, xv_sb, 3),
            (nc.sync, xv_d, xv_sb, 4), (nc.scalar, xv_d, xv_sb, 5),
            (nc.sync, xv_d, xv_sb, 6), (nc.scalar, xv_d, xv_sb, 7),
        ]:
            in_chunk(eng, t_d, t_sb, j)

        # preload the exp spline table while the inputs stream
        nc.scalar.activation(out=warm_sb[:], in_=ones_sb[:], func=EXP)

        # --- just-in-time projections (vt quads + kg groups) ---
        def emit_vt_quad(q):
            ps_q = pr_ps.tile([MC, NG], F32, tag="p", name=f"ps_vtq{q}")
            for j in range(4):
                mcols = slice((4 * q + j) * MC, (4 * q + j + 1) * MC)
                nc.tensor.matmul(out=ps_q[:, j * MC:(j + 1) * MC],
                                 lhsT=xv_sb[:, mcols], rhs=wv_sb[:])
            vcols = slice(q * 4 * MC, (q + 1) * 4 * MC)
            nc.vector.tensor_copy(vt_sb[:, vcols], ps_q[:])

        def emit_kg(g):
            ncols = slice(g * NG, (g + 1) * NG)
            ps_kg = pr_ps.tile([CH, NG], F32, tag="p", name=f"ps_kg{g}")
            nc.tensor.matmul(out=ps_kg[:], lhsT=gt_sb[:], rhs=xq_sb[:, ncols])
            nc.vector.tensor_copy(kg_sb[:, ncols], ps_kg[:])

        # vt quad q first used by AV of round k (see schedule analysis);
        # kg g needed by the first energy round of group g. kg projections
        # are emitted BEFORE the round's energy matmuls (round 0's energy
        # reads kg group 0); vt quads after (their xv DMA lands later and
        # must not head-of-line-block the energy stream on the in-order PE).
        vt_due = {0: [0], 1: [1], 2: [2], 4: [3], 5: [4], 6: [5],
                  8: [6], 9: [7]}
        kg_due = {0: [0]}
        for g in range(1, NGROUPS):
            kg_due.setdefault(11 * g - 2, []).append(g)

        # --- main attention loop, software-pipelined by one round ---
        rounds = []
        for g in range(NGROUPS):
            mi = 0
            for msz in MGROUPS:
                rounds.append((g, mi, msz))
                mi += msz

        o_tiles = {}

        def emit_out_round(g, mi, msz, ex_a, ex_d):
            if g not in o_tiles:
                o_tiles[g] = po_ps.tile([MC, NG], F32, tag="o", name=f"o_ps{g}")
            o_ps = o_tiles[g]
            for j in range(msz):
                vcols = slice((mi + j) * MC, (mi + j + 1) * MC)
                rhs = ex_d[:] if (ex_d is not None and j == msz - 1) \
                    else ex_a[:, j * NG:(j + 1) * NG]
                nc.tensor.matmul(
                    out=o_ps[:],
                    lhsT=vt_sb[:, vcols],
                    rhs=rhs,
                    start=(mi + j == 0),
                    stop=(mi + j == MCHUNKS - 1),
                )
            if mi + msz == MCHUNKS:
                emit_epilogue(g, o_ps)

        def emit_epilogue(g, o_ps):
            # o_ps row 0 = softmax denominator, rows 64..127 = channels
            ncols = slice(g * NG, (g + 1) * NG)
            last = g == NGROUPS - 1
            if last:
                # PE is nearly idle by now: rank-1 matmul broadcast beats the
                # DRAM bounce's ~4us latency on the final (unhidden) epilogue.
                # recip straight to bf16, broadcast on the PE, multiply reads
                # both operands from PSUM, adds run on GPSIMD in parallel.
                rec = small.tile([1, NG], F32, tag="rec", name=f"rec{g}")
                nc.vector.reciprocal_approx_fast(out=rec[:], in_=o_ps[0:1, :])
                rec_b = small.tile([1, NG], BF16, tag="recw", name=f"recw{g}")
                nc.vector.tensor_copy(rec_b[:], rec[:])
                bc_ps = pr_ps.tile([C, NG], F32, tag="p", name=f"bc{g}")
                nc.tensor.matmul(out=bc_ps[:], lhsT=ones_sb[:], rhs=rec_b[:])
                rec_bc = small.tile([C, NG], F32, tag="recb", name=f"recb{g}")
                half = NG // 2
                for h, eng in ((0, nc.sync), (1, nc.scalar)):
                    chs = slice(g * NG + h * half, g * NG + (h + 1) * half)
                    ph = slice(h * half, (h + 1) * half)
                    nc.vector.tensor_copy(rec_bc[:, ph], bc_ps[:, ph])
                    nc.vector.tensor_mul(out_sb[:, chs], o_ps[C:MC, ph],
                                         rec_bc[:, ph])
                    nc.gpsimd.tensor_add(out_sb[:, chs], out_sb[:, chs],
                                         xv_sb[0:C, chs])
                    eng.dma_start(out=out_d[:, chs], in_=out_sb[:, chs])
            else:
                rec = small.tile([1, NG], F32, tag="rec", name=f"rec{g}")
                nc.vector.reciprocal_approx_fast(out=rec[:], in_=o_ps[0:1, :])
                rec_bc = small.tile([C, NG], F32, tag="recb", name=f"recb{g}")
                # copy the numerators out of PSUM so the single o_ps bank is
                # free for the next group before its first AV arrives
                onum = small.tile([C, NG], F32, tag="onum", name=f"onum{g}")
                nc.vector.tensor_copy(onum[:], o_ps[C:MC, :])
                # broadcast 1/den via DRAM bounce (partition-stride-0 read);
                # latency hides inside the pipeline
                nc.sync.dma_start(out=rec_d[g:g + 1, :], in_=rec[:])
                rd = rec_d[g:g + 1, :]
                rec_bcast = bass.AP(tensor=rd.tensor, offset=rd.offset,
                                    ap=[[0, C]] + list(rd.ap[1:]))
                nc.sync.dma_start(out=rec_bc[:], in_=rec_bcast)
                # mul/add on GPSIMD: its queue can absorb the bounce latency
                # without blocking the DVE exp stream
                nc.gpsimd.tensor_mul(out_sb[:, ncols], onum[:], rec_bc[:])
                nc.gpsimd.tensor_add(out_sb[:, ncols], out_sb[:, ncols],
                                     xv_sb[0:C, ncols])
                nc.sync.dma_start(out=out_d[:, ncols], in_=out_sb[:, ncols])

        pending = None
        for ridx, (g, mi, msz) in enumerate(rounds):
            ncols = slice(g * NG, (g + 1) * NG)
            for idx in kg_due.get(ridx, []):
                emit_kg(idx)
            # energy psum: ACT chunks and the DVE chunk live in separate
            # pools so the ping-pong reuse chains are independent
            # rounds 0-1 keep all-ACT exp: the DVE is on the critical path
            # there copying kg0/vtq0 out of the projection psum
            dve_last = (msz == RSZ or ridx == len(rounds) - 1) and ridx >= 2
            na = msz - 1 if (dve_last and msz == 2) else min(msz, 2)
            e_a = pa_ps.tile([MC, 2 * NG], F32, tag="e", name=f"e_a{ridx}")
            e_d = pd_ps.tile([MC, NG], F32, tag="d", name=f"e_d{ridx}") \
                if (msz == RSZ or dve_last) else None
            ex_a = expp.tile([MC, 2 * NG], BF16, tag="ex", name=f"ex{ridx}")
            ex_d = expd.tile([MC, NG], BF16, tag="xd", name=f"xd{ridx}") \
                if dve_last else None
            for j in range(msz):
                mcols = slice((mi + j) * MC, (mi + j + 1) * MC)
                tgt = e_d[:] if (e_d is not None and j == msz - 1) \
                    else e_a[:, j * NG:(j + 1) * NG]
                nc.tensor.matmul(out=tgt, lhsT=xq_sb[:, mcols],
                                 rhs=kg_sb[:, ncols])
            # exp: ACT exact spline on the first chunks; DVE Schraudolph on
            # the last chunk for groups >= 1
            nc.scalar.activation(out=ex_a[:, :na * NG], in_=e_a[:, :na * NG],
                                 func=EXP)
            if msz == RSZ and not dve_last:
                # group 0: third chunk also on ACT (DVE is busy projecting)
                ex3 = expd.tile([MC, NG], BF16, tag="xd", name=f"xd{ridx}")
                nc.scalar.activation(out=ex3[:], in_=e_d[:], func=EXP)
                ex_d = ex3
            if dve_last:
                nc.vector.tensor_scalar(
                    out=ex_d[:].bitcast(I16),
                    in0=e_d[:],
                    scalar1=EXPA, scalar2=EXPB, op0=MULT, op1=ADD,
                )
            for idx in vt_due.get(ridx, []):
                emit_vt_quad(idx)
            if pending is not None:
                emit_out_round(*pending)
            pending = (g, mi, msz, ex_a, ex_d)
        emit_out_round(*pending)

    nc.finalize()
    return nc


def get_program():
    if "nc" not in _CACHE:
        _CACHE["nc"] = build_program()
    return _CACHE["nc"]


def prep_inputs(query, value, Wq, bq, Wk, bk, Wv, bv):
    B = query.shape[0]
    ones = np.ones((B, 1, N), np.float32)
    xq = np.concatenate([query.reshape(B, C, N).astype(np.float32), ones],
                        axis=1).astype(ml_dtypes.bfloat16)
    xv = np.concatenate([value.reshape(B, C, N).astype(np.float32), ones],
                        axis=1).astype(ml_dtypes.bfloat16)
    wq_aug = np.concatenate([Wq.T, bq[None, :]], axis=0).astype(np.float64)
    wk_aug = np.concatenate([Wk.T, bk[None, :]], axis=0).astype(np.float64)
    gt = (wq_aug @ wk_aug.T).astype(ml_dtypes.bfloat16)
    # columns: 0 = ones (denominator), 64..127 = projected channels
    wv_ = np.zeros((CH, MC), np.float32)
    wv_[C, 0] = 1.0
    wv_[:C, C:MC] = Wv.T
    wv_[C, C:MC] = bv
    wv_ = wv_.astype(ml_dtypes.bfloat16)
    return [
        {
            "xq": np.ascontiguousarray(xq[b]),
            "xv": np.ascontiguousarray(xv[b]),
            "gt": gt,
            "wv": wv_,
            "ones": np.ones((1, C), ml_dtypes.bfloat16),
        }
        for b in range(B)
    ]


def kernel(query, value, Wq, bq, Wk, bk, Wv, bv):
    query = np.asarray(query)
    value = np.asarray(value)
    B, _, H, W = query.shape
    in_maps = prep_inputs(
        query, value,
        np.asarray(Wq), np.asarray(bq), np.asarray(Wk),
        np.asarray(bk), np.asarray(Wv), np.asarray(bv),
    )
    nc = get_program()
    try:
        res = run_bass_kernel_spmd(nc, in_maps, core_ids=list(range(B)), trace=TRACE)
    except ModuleNotFoundError:
        res = run_bass_kernel_spmd(nc, in_maps, core_ids=list(range(B)), trace=False)
    _CACHE["last_result"] = res
    out = np.stack([res.results[b]["out"] for b in range(B)])
    return out.reshape(B, C, H, W).astype(query.dtype)
